# revision 10
# baseline (speedup 1.0000x reference)
"""Trainium2 Bass kernel for nn_graph_constructor (topk_masking).

Computes: adj = relu(tanh(3*(nv1@nv2.T - nv2@nv1.T))); per-row top-k of
(adj + 0.01*noise) masks adj; plus identity. Full [8192,8192] in/out.

Strategy (8 NeuronCores, row-sharded):
  - host: nv1/nv2 projections (tiny), pack X=[nv1|-nv2], W=[nv2|nv1] so the
    antisymmetric score block is ONE K=128 fp32 matmul per output tile.
  - device (per core, 1024 rows = 8 tiles of 128 partitions):
      PE:   a = X_blk @ W.T           (16 psum chunks of 512)
      ACT:  tv = tanh(3*a), ns = 0.01*noise
      DVE:  s = tv + ns; per-256-chunk top-8 candidates (InstMax);
            4 rounds max+match_replace on candidates -> threshold t (=kth)
      POOL: mask = (s >= t); out = mask * tv
      DMA:  noise in, out rows out (memory-bound: ~64MiB/core)
  - host: exact tie trim at the threshold (jax top_k keeps lowest index),
    full-row recompute fallback for any anomalous row, add identity.
"""

import numpy as np
from contextlib import ExitStack

import concourse.bass as bass
import concourse.bacc as bacc
import concourse.mybir as mybir
from concourse.tile import TileContext
from concourse.bass_utils import run_bass_kernel_spmd

ALPHA = 3.0
N = 8192
DIM = 64
CORES = 8
RPC = N // CORES          # rows per core
P = 128                   # partitions / tile rows
TILES = RPC // P          # row tiles per core
NBLK = 512                # matmul free-dim chunk
CHUNK = 256               # stage-1 candidate chunk
NCH = N // CHUNK          # 32 chunks -> 256 candidates/row
F32 = mybir.dt.float32
NEG = -1.0e30

_prog_cache: dict = {}


def _build_program(k: int) -> bass.Bass:
    rounds = (k + 7) // 8
    assert rounds * 8 <= NCH * 8, "k too large for candidate set"
    nc = bacc.Bacc("TRN2", target_bir_lowering=False, debug=False,
                   num_devices=CORES)
    # xt (K=128 x RPC lhsT block) and wt (K=128 x N rhs) packed in one
    # tensor: ONE dma -> ONE semaphore (PE Matmult allows a single wait).
    wx_d = nc.dram_tensor("wx", [P, RPC + N], F32, kind="ExternalInput").ap()
    nz_d = nc.dram_tensor("noise", [RPC, N], F32, kind="ExternalInput").ap()
    out_d = nc.dram_tensor("out", [RPC, N], F32, kind="ExternalOutput").ap()

    with TileContext(nc) as tc, ExitStack() as ctx:
        const_pool = ctx.enter_context(tc.tile_pool(name="const", bufs=1))
        a_pool = ctx.enter_context(tc.tile_pool(name="apool", bufs=3))
        b_pool = ctx.enter_context(tc.tile_pool(name="bpool", bufs=2))
        c_pool = ctx.enter_context(tc.tile_pool(name="cpool", bufs=2))
        m_pool = ctx.enter_context(tc.tile_pool(name="mpool", bufs=2))
        ps_pool = ctx.enter_context(
            tc.tile_pool(name="psum", bufs=4, space="PSUM"))

        wx_sb = const_pool.tile([P, RPC + N], F32)
        nc.sync.dma_start(wx_sb[:], wx_d[:])

        for m in range(TILES):
            # noise rows for this tile; buffer A is reused in place:
            # noise -> ns -> s -> mask -> out
            A = a_pool.tile([P, N], F32, tag="A")
            nc.sync.dma_start(A[:], nz_d[m * P:(m + 1) * P, :])

            # tv = tanh(3 * (X_blk @ W.T)), chunked through PSUM
            B = b_pool.tile([P, N], F32, tag="B")
            for n in range(N // NBLK):
                ps = ps_pool.tile([P, NBLK], F32, tag="ps")
                nc.tensor.matmul(ps[:], wx_sb[:, m * P:(m + 1) * P],
                                 wx_sb[:, RPC + n * NBLK:RPC + (n + 1) * NBLK],
                                 start=True, stop=True)
                nc.scalar.activation(B[:, n * NBLK:(n + 1) * NBLK], ps[:],
                                     mybir.ActivationFunctionType.Tanh,
                                     bias=0.0, scale=ALPHA)

            # ns = 0.01 * noise  (ACT, in place)
            nc.scalar.activation(A[:], A[:],
                                 mybir.ActivationFunctionType.Copy,
                                 bias=0.0, scale=0.01)
            # s = tv + ns  (DVE, in place into A)
            nc.vector.tensor_add(A[:], A[:], B[:])

            # stage 1: top-8 per 256-chunk -> 256 candidates
            cand = c_pool.tile([P, NCH * 8], F32, tag="cand")
            for c in range(NCH):
                nc.vector.max(cand[:, c * 8:(c + 1) * 8],
                              A[:, c * CHUNK:(c + 1) * CHUNK])

            # stage 2: iterative top-8 of candidates -> kth largest
            maxs = m_pool.tile([P, rounds * 8], F32, tag="maxs")
            for r in range(rounds):
                ms = maxs[:, r * 8:(r + 1) * 8]
                nc.vector.max(ms, cand[:])
                if r < rounds - 1:
                    nc.vector.match_replace(cand[:], ms, cand[:], NEG)
            last = (rounds - 1) * 8 + (k - 1) % 8
            t_ap = maxs[:, last:last + 1]

            # mask & apply on POOL (gpsimd): A = (s >= t); A = A * tv
            nc.gpsimd.tensor_scalar(A[:], A[:], t_ap, None,
                                    op0=mybir.AluOpType.is_ge)
            nc.gpsimd.tensor_tensor(A[:], A[:], B[:],
                                    op=mybir.AluOpType.mult)

            nc.sync.dma_start(out_d[m * P:(m + 1) * P, :], A[:])
    nc.finalize()
    return nc


def get_program(k: int) -> bass.Bass:
    if k not in _prog_cache:
        _prog_cache[k] = _build_program(k)
    return _prog_cache[k]


def _host_nv(idx, emb1, emb2, lin1_w, lin1_b, lin2_w, lin2_b):
    idx = np.asarray(idx)
    e1 = np.asarray(emb1, dtype=np.float32)[idx]
    e2 = np.asarray(emb2, dtype=np.float32)[idx]
    nv1 = np.tanh(ALPHA * (e1 @ np.asarray(lin1_w, np.float32).T
                           + np.asarray(lin1_b, np.float32))).astype(np.float32)
    nv2 = np.tanh(ALPHA * (e2 @ np.asarray(lin2_w, np.float32).T
                           + np.asarray(lin2_b, np.float32))).astype(np.float32)
    return nv1, nv2


def _row_reference(nv1, nv2, noise_row, r, k):
    """Exact host recompute of one output row (pre-identity)."""
    x = np.concatenate([nv1[r], -nv2[r]]).astype(np.float32)
    a = (np.concatenate([nv2, nv1], axis=1).astype(np.float32) @ x).astype(np.float32)
    tv = np.tanh(ALPHA * a).astype(np.float32)
    adj = np.maximum(tv, np.float32(0.0))
    s = (adj + noise_row * np.float32(0.01)).astype(np.float32)
    order = np.argsort(-s, kind="stable")[:k]
    row = np.zeros(N, np.float32)
    row[order] = adj[order]
    return row


def kernel(idx, emb1, emb2, lin1_w, lin1_b, lin2_w, lin2_b, noise, k,
           _trace=False):
    k = int(k)
    noise = np.ascontiguousarray(np.asarray(noise, dtype=np.float32))
    nv1, nv2 = _host_nv(idx, emb1, emb2, lin1_w, lin1_b, lin2_w, lin2_b)

    X = np.concatenate([nv1, -nv2], axis=1).astype(np.float32)   # [N, 128]
    W = np.concatenate([nv2, nv1], axis=1).astype(np.float32)    # [N, 128]
    XT = np.ascontiguousarray(X.T)                               # [128, N]
    WT = np.ascontiguousarray(W.T)                               # [128, N]

    nc = get_program(k)
    in_maps = [{
        "wx": np.ascontiguousarray(
            np.concatenate([XT[:, c * RPC:(c + 1) * RPC], WT], axis=1)),
        "noise": np.ascontiguousarray(noise[c * RPC:(c + 1) * RPC]),
    } for c in range(CORES)]

    res = run_bass_kernel_spmd(nc, in_maps, core_ids=list(range(CORES)),
                               trace=_trace)
    out = np.concatenate([res.results[c]["out"] for c in range(CORES)], axis=0)

    # --- host fixup: exact top-k semantics at the boundary ---
    cnt = np.count_nonzero(out, axis=1)
    neg = (out.min(axis=1) < 0)
    bad_rows = np.flatnonzero((cnt != k) | neg)
    for r in bad_rows:
        if cnt[r] > k and not neg[r]:
            # tie at the threshold: keep lowest-index entries (jax top_k)
            sel = np.flatnonzero(out[r])
            s_sel = (out[r, sel] + np.float32(0.01) * noise[r, sel]).astype(np.float32)
            t = s_sel.min()
            tied = sel[s_sel == t]
            excess = sel.size - k
            if excess <= tied.size:
                out[r, tied[tied.size - excess:]] = 0.0
            else:
                out[r] = _row_reference(nv1, nv2, noise[r], r, k)
        else:
            out[r] = _row_reference(nv1, nv2, noise[r], r, k)

    out[np.arange(N), np.arange(N)] += np.float32(1.0)
    if _trace:
        return out, res
    return out


# revision 11
# speedup vs baseline: 4.2552x; 4.2552x over previous
"""Trainium2 Bass kernel for nn_graph_constructor (topk_masking).

Computes: adj = relu(tanh(3*(nv1@nv2.T - nv2@nv1.T))); per-row top-k of
(adj + 0.01*noise) masks adj; plus identity. Full [8192,8192] in/out.

Strategy (8 NeuronCores, row-sharded):
  - host: nv1/nv2 projections (tiny), pack X=[nv1|-nv2], W=[nv2|nv1] so the
    antisymmetric score block is ONE K=128 fp32 matmul per output tile.
  - device (per core, 1024 rows = 8 tiles of 128 partitions):
      PE:   a = X_blk @ W.T              (psum chunks)
      ACT:  tv = tanh(3*a); ns = 0.01*noise; final out' = relu(s - t_{k+1})
      DVE:  s = tv + ns; per-256-chunk top-8 candidates (InstMax);
            5 rounds max+match_replace on candidates -> (k+1)-th largest
      DMA:  noise in, out' rows out (memory-bound: ~64MiB/core)
    out'[i,j] = relu(s[i,j] - t_{k+1}[i]) is > 0 exactly on the top-k set
    (when t_k > t_{k+1}; boundary ties give < k positives -> host fallback).
  - host: mask = out' > 0; selected values recomputed exactly as
    tanh(3 * <X[r], W[c]>) (saturated tanh makes rounding immaterial);
    rare tie rows recomputed fully; add identity.

GpSimd is deliberately unused for elementwise work: measured ~123us per
[128,8192] tensor_scalar AND its SBUF traffic starves concurrent DVE ~10x.
"""

import numpy as np
from contextlib import ExitStack

import concourse.bass as bass
import concourse.bacc as bacc
import concourse.mybir as mybir
from concourse.tile import TileContext
from concourse.bass_utils import run_bass_kernel_spmd

ALPHA = 3.0
N = 8192
DIM = 64
CORES = 8
RPC = N // CORES          # rows per core
P = 128                   # partitions / tile rows
TILES = RPC // P          # row tiles per core
NBLK = 512                # matmul free-dim chunk (one PSUM bank)
PSB = 1024                # psum tile width (2 banks, 2 matmuls, 1 ACT pass)
CHUNK = 256               # stage-1 candidate chunk
NCH = N // CHUNK          # 32 chunks -> 256 candidates/row
F32 = mybir.dt.float32
NEG = -1.0e30

_prog_cache: dict = {}


def _build_program(k: int) -> bass.Bass:
    rank = k + 1                       # extract the (k+1)-th largest
    rounds = (rank + 7) // 8
    last_col = (rank - 1) % 8
    assert rounds * 8 <= NCH * 8

    nc = bacc.Bacc("TRN2", target_bir_lowering=False, debug=False,
                   num_devices=CORES)
    # xt (K=128 x RPC lhsT block) and wt (K=128 x N rhs) packed in one
    # tensor: ONE dma -> ONE semaphore (PE Matmult allows a single wait).
    wx_d = nc.dram_tensor("wx", [P, RPC + N], F32, kind="ExternalInput").ap()
    nz_d = nc.dram_tensor("noise", [RPC, N], F32, kind="ExternalInput").ap()
    out_d = nc.dram_tensor("out", [RPC, N], F32, kind="ExternalOutput").ap()

    with TileContext(nc) as tc, ExitStack() as ctx:
        const_pool = ctx.enter_context(tc.tile_pool(name="const", bufs=1))
        a_pool = ctx.enter_context(tc.tile_pool(name="apool", bufs=3))
        b_pool = ctx.enter_context(tc.tile_pool(name="bpool", bufs=2))
        c_pool = ctx.enter_context(tc.tile_pool(name="cpool", bufs=2))
        m_pool = ctx.enter_context(tc.tile_pool(name="mpool", bufs=2))
        ps_pool = ctx.enter_context(
            tc.tile_pool(name="psum", bufs=4, space="PSUM"))

        wx_sb = const_pool.tile([P, RPC + N], F32)
        nc.sync.dma_start(wx_sb[:], wx_d[:])

        for m in range(TILES):
            # noise rows for this tile; buffer A is reused in place:
            # noise -> ns -> s -> out'
            A = a_pool.tile([P, N], F32, tag="A")
            nc.sync.dma_start(A[:], nz_d[m * P:(m + 1) * P, :])

            # tv = tanh(3 * (X_blk @ W.T)); two matmuls per 2-bank psum tile
            B = b_pool.tile([P, N], F32, tag="B")
            for nb in range(N // PSB):
                ps = ps_pool.tile([P, PSB], F32, tag="ps")
                for h in range(PSB // NBLK):
                    off = RPC + nb * PSB + h * NBLK
                    nc.tensor.matmul(ps[:, h * NBLK:(h + 1) * NBLK],
                                     wx_sb[:, m * P:(m + 1) * P],
                                     wx_sb[:, off:off + NBLK],
                                     start=True, stop=True)
                nc.scalar.activation(B[:, nb * PSB:(nb + 1) * PSB], ps[:],
                                     mybir.ActivationFunctionType.Tanh,
                                     bias=0.0, scale=ALPHA)

            # ns = 0.01 * noise  (ACT, in place)
            nc.scalar.activation(A[:], A[:],
                                 mybir.ActivationFunctionType.Copy,
                                 bias=0.0, scale=0.01)
            # s = tv + ns  (DVE, in place into A)
            nc.vector.tensor_add(A[:], A[:], B[:])

            # stage 1: top-8 per 256-chunk -> 256 candidates
            cand = c_pool.tile([P, NCH * 8], F32, tag="cand")
            for c in range(NCH):
                nc.vector.max(cand[:, c * 8:(c + 1) * 8],
                              A[:, c * CHUNK:(c + 1) * CHUNK])

            # stage 2: iterative top-8 of candidates -> (k+1)-th largest
            maxs = m_pool.tile([P, rounds * 8], F32, tag="maxs")
            for r in range(rounds):
                ms = maxs[:, r * 8:(r + 1) * 8]
                nc.vector.max(ms, cand[:])
                if r < rounds - 1:
                    nc.vector.match_replace(cand[:], ms, cand[:], NEG)
            t_ap = maxs[:, rounds * 8 - 8 + last_col:rounds * 8 - 8 + last_col + 1]
            neg_t = m_pool.tile([P, 1], F32, tag="negt")
            nc.vector.tensor_scalar_mul(neg_t[:], t_ap, -1.0)

            # out' = relu(s - t_{k+1})  (ACT with per-partition bias)
            nc.scalar.activation(A[:], A[:],
                                 mybir.ActivationFunctionType.Relu,
                                 bias=neg_t[:, 0:1], scale=1.0)

            nc.sync.dma_start(out_d[m * P:(m + 1) * P, :], A[:])
    nc.finalize()
    return nc


def get_program(k: int) -> bass.Bass:
    if k not in _prog_cache:
        _prog_cache[k] = _build_program(k)
    return _prog_cache[k]


def _host_nv(idx, emb1, emb2, lin1_w, lin1_b, lin2_w, lin2_b):
    idx = np.asarray(idx)
    e1 = np.asarray(emb1, dtype=np.float32)[idx]
    e2 = np.asarray(emb2, dtype=np.float32)[idx]
    nv1 = np.tanh(ALPHA * (e1 @ np.asarray(lin1_w, np.float32).T
                           + np.asarray(lin1_b, np.float32))).astype(np.float32)
    nv2 = np.tanh(ALPHA * (e2 @ np.asarray(lin2_w, np.float32).T
                           + np.asarray(lin2_b, np.float32))).astype(np.float32)
    return nv1, nv2


def _row_reference(X, W, noise_row, r, k):
    """Exact host recompute of one output row (pre-identity)."""
    a = (W @ X[r]).astype(np.float32)
    tv = np.tanh(ALPHA * a).astype(np.float32)
    adj = np.maximum(tv, np.float32(0.0))
    s = (adj + noise_row * np.float32(0.01)).astype(np.float32)
    order = np.argsort(-s, kind="stable")[:k]
    row = np.zeros(N, np.float32)
    row[order] = adj[order]
    return row


def kernel(idx, emb1, emb2, lin1_w, lin1_b, lin2_w, lin2_b, noise, k,
           _trace=False):
    k = int(k)
    noise = np.ascontiguousarray(np.asarray(noise, dtype=np.float32))
    nv1, nv2 = _host_nv(idx, emb1, emb2, lin1_w, lin1_b, lin2_w, lin2_b)

    X = np.concatenate([nv1, -nv2], axis=1).astype(np.float32)   # [N, 128]
    W = np.concatenate([nv2, nv1], axis=1).astype(np.float32)    # [N, 128]
    XT = np.ascontiguousarray(X.T)                               # [128, N]
    WT = np.ascontiguousarray(W.T)                               # [128, N]

    nc = get_program(k)
    in_maps = [{
        "wx": np.ascontiguousarray(
            np.concatenate([XT[:, c * RPC:(c + 1) * RPC], WT], axis=1)),
        "noise": np.ascontiguousarray(noise[c * RPC:(c + 1) * RPC]),
    } for c in range(CORES)]

    res = run_bass_kernel_spmd(nc, in_maps, core_ids=list(range(CORES)),
                               trace=_trace)
    op = np.concatenate([res.results[c]["out"] for c in range(CORES)], axis=0)

    # --- host: mask = out' > 0, exact value reconstruction, tie fallback ---
    mask = op > 0
    cnt = mask.sum(axis=1)
    bad_rows = np.flatnonzero(cnt != k)
    mask[bad_rows] = False

    rows, cols = np.nonzero(mask)
    vals = np.tanh(ALPHA * np.einsum("ij,ij->i", X[rows], W[cols])
                   ).astype(np.float32)
    out = np.zeros((N, N), np.float32)
    out[rows, cols] = np.maximum(vals, np.float32(0.0))
    for r in bad_rows:
        out[r] = _row_reference(X, W, noise[r], r, k)

    out[np.arange(N), np.arange(N)] += np.float32(1.0)
    if _trace:
        return out, res
    return out


# revision 13
# speedup vs baseline: 4.4313x; 1.0414x over previous
"""Trainium2 Bass kernel for nn_graph_constructor (topk_masking).

Computes: adj = relu(tanh(3*(nv1@nv2.T - nv2@nv1.T))); per-row top-k of
(adj + 0.01*noise) masks adj; plus identity. Full [8192,8192] in/out.

Strategy (8 NeuronCores, row-sharded):
  - host: nv1/nv2 projections (tiny), pack X=[nv1|-nv2], W=[nv2|nv1] so the
    antisymmetric score block is ONE K=128 fp32 matmul per output tile.
  - device (per core, 1024 rows = 8 tiles of 128 partitions):
      PE:   a = X_blk @ W.T              (psum chunks)
      ACT:  tv = tanh(3*a); ns = 0.01*noise; final out' = relu(s - t_{k+1})
      DVE:  s = tv + ns; per-256-chunk top-8 candidates (InstMax);
            5 rounds max+match_replace on candidates -> (k+1)-th largest
      DMA:  noise in, out' rows out (memory-bound: ~64MiB/core)
    out'[i,j] = relu(s[i,j] - t_{k+1}[i]) is > 0 exactly on the top-k set
    (when t_k > t_{k+1}; boundary ties give < k positives -> host fallback).
  - host: mask = out' > 0; selected values recomputed exactly as
    tanh(3 * <X[r], W[c]>) (saturated tanh makes rounding immaterial);
    rare tie rows recomputed fully; add identity.

GpSimd is deliberately unused for elementwise work: measured ~123us per
[128,8192] tensor_scalar AND its SBUF traffic starves concurrent DVE ~10x.
"""

import numpy as np
from contextlib import ExitStack

import concourse.bass as bass
import concourse.bacc as bacc
import concourse.mybir as mybir
from concourse.tile import TileContext
from concourse.bass_utils import run_bass_kernel_spmd

ALPHA = 3.0
N = 8192
DIM = 64
CORES = 8
RPC = N // CORES          # rows per core
P = 128                   # partitions / tile rows
TILES = RPC // P          # row tiles per core
NBLK = 512                # matmul free-dim chunk (one PSUM bank)
PSB = 1024                # psum tile width (2 banks, 2 matmuls, 1 ACT pass)
CHUNK = 256               # stage-1 candidate chunk
NCH = N // CHUNK          # 32 chunks -> 256 candidates/row
F32 = mybir.dt.float32
NEG = -1.0e30

_prog_cache: dict = {}


def _build_program(k: int) -> bass.Bass:
    rounds = (k + 7) // 8              # extract the k-th largest
    last_col = (k - 1) % 8
    assert rounds * 8 <= NCH * 8

    nc = bacc.Bacc("TRN2", target_bir_lowering=False, debug=False,
                   num_devices=CORES)
    # xt (K=128 x RPC lhsT block) and wt (K=128 x N rhs) packed in one
    # tensor: ONE dma -> ONE semaphore (PE Matmult allows a single wait).
    wx_d = nc.dram_tensor("wx", [P, RPC + N], F32, kind="ExternalInput").ap()
    nz_d = nc.dram_tensor("noise", [RPC, N], F32, kind="ExternalInput").ap()
    out_d = nc.dram_tensor("out", [RPC, N], F32, kind="ExternalOutput").ap()

    with TileContext(nc) as tc, ExitStack() as ctx:
        const_pool = ctx.enter_context(tc.tile_pool(name="const", bufs=1))
        a_pool = ctx.enter_context(tc.tile_pool(name="apool", bufs=5))
        c_pool = ctx.enter_context(tc.tile_pool(name="cpool", bufs=2))
        m_pool = ctx.enter_context(tc.tile_pool(name="mpool", bufs=2))
        ps_pool = ctx.enter_context(
            tc.tile_pool(name="psum", bufs=2, space="PSUM"))
        pt_pool = ctx.enter_context(
            tc.tile_pool(name="psum_tv", bufs=2, space="PSUM"))

        wx_sb = const_pool.tile([P, RPC + N], F32)
        nc.sync.dma_start(wx_sb[:], wx_d[:])

        for m in range(TILES):
            # noise rows for this tile; buffer A is reused in place:
            # noise -> ns -> s -> out'
            A = a_pool.tile([P, N], F32, tag="A")
            nc.sync.dma_start(A[:], nz_d[m * P:(m + 1) * P, :])

            # ns = 0.01 * noise  (ACT, in place)
            nc.scalar.activation(A[:], A[:],
                                 mybir.ActivationFunctionType.Copy,
                                 bias=0.0, scale=0.01)

            # tv stays in PSUM: a -> tanh -> ps_tv; DVE adds it into A
            for nb in range(N // PSB):
                ps = ps_pool.tile([P, PSB], F32, tag="ps")
                for h in range(PSB // NBLK):
                    off = RPC + nb * PSB + h * NBLK
                    nc.tensor.matmul(ps[:, h * NBLK:(h + 1) * NBLK],
                                     wx_sb[:, m * P:(m + 1) * P],
                                     wx_sb[:, off:off + NBLK],
                                     start=True, stop=True)
                ps_tv = pt_pool.tile([P, PSB], F32, tag="pstv")
                nc.scalar.activation(ps_tv[:], ps[:],
                                     mybir.ActivationFunctionType.Tanh,
                                     bias=0.0, scale=ALPHA)
                # s chunk = ns chunk + tv chunk  (DVE, in place into A)
                nc.vector.tensor_add(A[:, nb * PSB:(nb + 1) * PSB],
                                     A[:, nb * PSB:(nb + 1) * PSB], ps_tv[:])

            # stage 1: top-8 per 256-chunk -> 256 candidates
            cand = c_pool.tile([P, NCH * 8], F32, tag="cand")
            for c in range(NCH):
                nc.vector.max(cand[:, c * 8:(c + 1) * 8],
                              A[:, c * CHUNK:(c + 1) * CHUNK])

            # stage 2: iterative top-8 of candidates -> k-th largest
            maxs = m_pool.tile([P, rounds * 8], F32, tag="maxs")
            for r in range(rounds):
                ms = maxs[:, r * 8:(r + 1) * 8]
                nc.vector.max(ms, cand[:])
                if r < rounds - 1:
                    nc.vector.match_replace(cand[:], ms, cand[:], NEG)
            t_ap = maxs[:, rounds * 8 - 8 + last_col:rounds * 8 - 8 + last_col + 1]
            neg_t = m_pool.tile([P, 1], F32, tag="negt")
            nc.vector.tensor_scalar_mul(neg_t[:], t_ap, -1.0)

            # out' = s - t_k  (ACT Identity with per-partition bias; signed.
            # >0 above threshold, ==0 exactly on tied boundary, <0 below)
            nc.scalar.activation(A[:], A[:],
                                 mybir.ActivationFunctionType.Identity,
                                 bias=neg_t[:, 0:1], scale=1.0)

            nc.sync.dma_start(out_d[m * P:(m + 1) * P, :], A[:])
    nc.finalize()
    return nc


def get_program(k: int) -> bass.Bass:
    if k not in _prog_cache:
        _prog_cache[k] = _build_program(k)
    return _prog_cache[k]


def _host_nv(idx, emb1, emb2, lin1_w, lin1_b, lin2_w, lin2_b):
    idx = np.asarray(idx)
    e1 = np.asarray(emb1, dtype=np.float32)[idx]
    e2 = np.asarray(emb2, dtype=np.float32)[idx]
    nv1 = np.tanh(ALPHA * (e1 @ np.asarray(lin1_w, np.float32).T
                           + np.asarray(lin1_b, np.float32))).astype(np.float32)
    nv2 = np.tanh(ALPHA * (e2 @ np.asarray(lin2_w, np.float32).T
                           + np.asarray(lin2_b, np.float32))).astype(np.float32)
    return nv1, nv2


def _row_reference(X, W, noise_row, r, k):
    """Exact host recompute of one output row (pre-identity)."""
    a = (W @ X[r]).astype(np.float32)
    tv = np.tanh(ALPHA * a).astype(np.float32)
    adj = np.maximum(tv, np.float32(0.0))
    s = (adj + noise_row * np.float32(0.01)).astype(np.float32)
    order = np.argsort(-s, kind="stable")[:k]
    row = np.zeros(N, np.float32)
    row[order] = adj[order]
    return row


def kernel(idx, emb1, emb2, lin1_w, lin1_b, lin2_w, lin2_b, noise, k,
           _trace=False):
    k = int(k)
    noise = np.ascontiguousarray(np.asarray(noise, dtype=np.float32))
    nv1, nv2 = _host_nv(idx, emb1, emb2, lin1_w, lin1_b, lin2_w, lin2_b)

    X = np.concatenate([nv1, -nv2], axis=1).astype(np.float32)   # [N, 128]
    W = np.concatenate([nv2, nv1], axis=1).astype(np.float32)    # [N, 128]
    XT = np.ascontiguousarray(X.T)                               # [128, N]
    WT = np.ascontiguousarray(W.T)                               # [128, N]

    nc = get_program(k)
    in_maps = [{
        "wx": np.ascontiguousarray(
            np.concatenate([XT[:, c * RPC:(c + 1) * RPC], WT], axis=1)),
        "noise": np.ascontiguousarray(noise[c * RPC:(c + 1) * RPC]),
    } for c in range(CORES)]

    res = run_bass_kernel_spmd(nc, in_maps, core_ids=list(range(CORES)),
                               trace=_trace)
    op = np.concatenate([res.results[c]["out"] for c in range(CORES)], axis=0)

    # --- host: mask = (s - t_k >= 0); ties sit exactly at 0 -> trim by
    # index (jax top_k keeps lowest indices); exact value reconstruction ---
    mask = op >= 0
    cnt = mask.sum(axis=1)
    full_rows = []
    for r in np.flatnonzero(cnt != k):
        if cnt[r] > k:
            tied = np.flatnonzero(op[r] == 0)
            excess = int(cnt[r]) - k
            if excess <= tied.size:
                mask[r, tied[tied.size - excess:]] = False
            else:
                mask[r] = False
                full_rows.append(r)
        else:
            mask[r] = False
            full_rows.append(r)

    rows, cols = np.nonzero(mask)
    vals = np.tanh(ALPHA * np.einsum("ij,ij->i", X[rows], W[cols])
                   ).astype(np.float32)
    out = np.zeros((N, N), np.float32)
    out[rows, cols] = np.maximum(vals, np.float32(0.0))
    for r in full_rows:
        out[r] = _row_reference(X, W, noise[r], r, k)

    out[np.arange(N), np.arange(N)] += np.float32(1.0)
    if _trace:
        return out, res
    return out


# revision 18
# speedup vs baseline: 5.2478x; 1.1843x over previous
"""Trainium2 Bass kernel for nn_graph_constructor (topk_masking).

Computes: adj = relu(tanh(3*(nv1@nv2.T - nv2@nv1.T))); per-row top-k of
(adj + 0.01*noise) masks adj; plus identity. Full [8192,8192] in/out.

Strategy (8 NeuronCores, row-sharded):
  - host: nv1/nv2 projections (tiny), pack X=[nv1|-nv2], W=[nv2|nv1] so the
    antisymmetric score block is ONE K=128 fp32 matmul per output tile.
  - device (per core, 1024 rows = 8 tiles of 128 partitions):
      PE:   a = X_blk @ W.T              (psum chunks)
      ACT:  tv = tanh(3*a); ns = 0.01*noise; final out' = relu(s - t_{k+1})
      DVE:  s = tv + ns; per-256-chunk top-8 candidates (InstMax);
            5 rounds max+match_replace on candidates -> (k+1)-th largest
      DMA:  noise in, out' rows out (memory-bound: ~64MiB/core)
    out'[i,j] = relu(s[i,j] - t_{k+1}[i]) is > 0 exactly on the top-k set
    (when t_k > t_{k+1}; boundary ties give < k positives -> host fallback).
  - host: mask = out' > 0; selected values recomputed exactly as
    tanh(3 * <X[r], W[c]>) (saturated tanh makes rounding immaterial);
    rare tie rows recomputed fully; add identity.

GpSimd is deliberately unused for elementwise work: measured ~123us per
[128,8192] tensor_scalar AND its SBUF traffic starves concurrent DVE ~10x.
"""

import numpy as np
from contextlib import ExitStack

import concourse.bass as bass
import concourse.bacc as bacc
import concourse.mybir as mybir
from concourse.tile import TileContext
from concourse.bass_utils import run_bass_kernel_spmd

ALPHA = 3.0
N = 8192
DIM = 64
CORES = 8
RPC = N // CORES          # rows per core
P = 128                   # partitions / tile rows
TILES = RPC // P          # row tiles per core
NBLK = 512                # matmul free-dim chunk (one PSUM bank)
PSB = 2048                # psum tile width (4 banks, 4 matmuls, 1 ACT pass)
CHUNK = 256               # stage-1 candidate chunk
NCH = N // CHUNK          # 32 chunks -> 256 candidates/row
F32 = mybir.dt.float32
NEG = -1.0e30

_prog_cache: dict = {}


def _build_program(k: int) -> bass.Bass:
    rounds = (k + 7) // 8              # extract the k-th largest
    last_col = (k - 1) % 8
    assert rounds * 8 <= NCH * 8

    nc = bacc.Bacc("TRN2", target_bir_lowering=False, debug=False,
                   num_devices=CORES)
    # xt (K=128 x RPC lhsT block) and wt (K=128 x N rhs) packed in one
    # tensor: ONE dma -> ONE semaphore (PE Matmult allows a single wait).
    wx_d = nc.dram_tensor("wx", [P, RPC + N], F32, kind="ExternalInput").ap()
    nz_d = nc.dram_tensor("noise", [RPC, N], F32, kind="ExternalInput").ap()
    out_d = nc.dram_tensor("out", [RPC, N], F32, kind="ExternalOutput").ap()

    with TileContext(nc) as tc, ExitStack() as ctx:
        const_pool = ctx.enter_context(tc.tile_pool(name="const", bufs=1))
        a_pool = ctx.enter_context(tc.tile_pool(name="apool", bufs=4))
        b_pool = ctx.enter_context(tc.tile_pool(name="bpool", bufs=3))
        c_pool = ctx.enter_context(tc.tile_pool(name="cpool", bufs=2))
        m_pool = ctx.enter_context(tc.tile_pool(name="mpool", bufs=2))
        ps_pool = ctx.enter_context(
            tc.tile_pool(name="psum", bufs=2, space="PSUM"))

        wx_sb = const_pool.tile([P, RPC + N], F32)
        nc.sync.dma_start(wx_sb[:], wx_d[:])

        for m in range(TILES):
            # pre-scaled noise (ns = 0.01*noise, scaled on host) for this
            # tile; buffer A is reused in place: ns -> s -> out'
            A = a_pool.tile([P, N], F32, tag="A")
            nc.sync.dma_start(A[:], nz_d[m * P:(m + 1) * P, :])

            # a -> tanh (psum -> sbuf bounce) -> add into A chunkwise
            for nb in range(N // PSB):
                ps = ps_pool.tile([P, PSB], F32, tag="ps")
                for h in range(PSB // NBLK):
                    off = RPC + nb * PSB + h * NBLK
                    nc.tensor.matmul(ps[:, h * NBLK:(h + 1) * NBLK],
                                     wx_sb[:, m * P:(m + 1) * P],
                                     wx_sb[:, off:off + NBLK],
                                     start=True, stop=True)
                bc = b_pool.tile([P, PSB], F32, tag="bc")
                nc.scalar.activation(bc[:], ps[:],
                                     mybir.ActivationFunctionType.Tanh,
                                     bias=0.0, scale=ALPHA)
                # s chunk = ns chunk + tv chunk  (DVE, in place into A)
                nc.vector.tensor_add(A[:, nb * PSB:(nb + 1) * PSB],
                                     A[:, nb * PSB:(nb + 1) * PSB], bc[:])

            # stage 1: top-8 per 256-chunk -> 256 candidates
            cand = c_pool.tile([P, NCH * 8], F32, tag="cand")
            for c in range(NCH):
                nc.vector.max(cand[:, c * 8:(c + 1) * 8],
                              A[:, c * CHUNK:(c + 1) * CHUNK])

            # stage 2: iterative top-8 of candidates -> k-th largest
            maxs = m_pool.tile([P, rounds * 8], F32, tag="maxs")
            for r in range(rounds):
                ms = maxs[:, r * 8:(r + 1) * 8]
                nc.vector.max(ms, cand[:])
                if r < rounds - 1:
                    nc.vector.match_replace(cand[:], ms, cand[:], NEG)
            t_ap = maxs[:, rounds * 8 - 8 + last_col:rounds * 8 - 8 + last_col + 1]
            neg_t = m_pool.tile([P, 1], F32, tag="negt")
            nc.vector.tensor_scalar_mul(neg_t[:], t_ap, -1.0)

            # out' = s - t_k  (ACT Identity with per-partition bias; signed.
            # >0 above threshold, ==0 exactly on tied boundary, <0 below)
            # Split in halves so out-DMA starts before the whole tile is done.
            H = N // 2
            for h in range(2):
                nc.scalar.activation(A[:, h * H:(h + 1) * H],
                                     A[:, h * H:(h + 1) * H],
                                     mybir.ActivationFunctionType.Identity,
                                     bias=neg_t[:, 0:1], scale=1.0)
                nc.sync.dma_start(out_d[m * P:(m + 1) * P, h * H:(h + 1) * H],
                                  A[:, h * H:(h + 1) * H])
    nc.finalize()
    return nc


def get_program(k: int) -> bass.Bass:
    if k not in _prog_cache:
        _prog_cache[k] = _build_program(k)
    return _prog_cache[k]


def _host_nv(idx, emb1, emb2, lin1_w, lin1_b, lin2_w, lin2_b):
    idx = np.asarray(idx)
    e1 = np.asarray(emb1, dtype=np.float32)[idx]
    e2 = np.asarray(emb2, dtype=np.float32)[idx]
    nv1 = np.tanh(ALPHA * (e1 @ np.asarray(lin1_w, np.float32).T
                           + np.asarray(lin1_b, np.float32))).astype(np.float32)
    nv2 = np.tanh(ALPHA * (e2 @ np.asarray(lin2_w, np.float32).T
                           + np.asarray(lin2_b, np.float32))).astype(np.float32)
    return nv1, nv2


def _row_reference(X, W, noise_row, r, k):
    """Exact host recompute of one output row (pre-identity)."""
    a = (W @ X[r]).astype(np.float32)
    tv = np.tanh(ALPHA * a).astype(np.float32)
    adj = np.maximum(tv, np.float32(0.0))
    s = (adj + noise_row * np.float32(0.01)).astype(np.float32)
    order = np.argsort(-s, kind="stable")[:k]
    row = np.zeros(N, np.float32)
    row[order] = adj[order]
    return row


def kernel(idx, emb1, emb2, lin1_w, lin1_b, lin2_w, lin2_b, noise, k,
           _trace=False):
    k = int(k)
    noise = np.ascontiguousarray(np.asarray(noise, dtype=np.float32))
    # ns = 0.01 * noise, f32 RNE — bit-identical to the reference's scaling.
    # Done while sharding; device memory traffic is unchanged (it still
    # streams the full block), this just drops one on-chip elementwise pass.
    ns = noise * np.float32(0.01)
    nv1, nv2 = _host_nv(idx, emb1, emb2, lin1_w, lin1_b, lin2_w, lin2_b)

    X = np.concatenate([nv1, -nv2], axis=1).astype(np.float32)   # [N, 128]
    W = np.concatenate([nv2, nv1], axis=1).astype(np.float32)    # [N, 128]
    XT = np.ascontiguousarray(X.T)                               # [128, N]
    WT = np.ascontiguousarray(W.T)                               # [128, N]

    nc = get_program(k)
    in_maps = [{
        "wx": np.ascontiguousarray(
            np.concatenate([XT[:, c * RPC:(c + 1) * RPC], WT], axis=1)),
        "noise": np.ascontiguousarray(ns[c * RPC:(c + 1) * RPC]),
    } for c in range(CORES)]

    res = run_bass_kernel_spmd(nc, in_maps, core_ids=list(range(CORES)),
                               trace=_trace)
    op = np.concatenate([res.results[c]["out"] for c in range(CORES)], axis=0)

    # --- host: mask = (s - t_k >= 0); ties sit exactly at 0 -> trim by
    # index (jax top_k keeps lowest indices); exact value reconstruction ---
    mask = op >= 0
    cnt = mask.sum(axis=1)
    full_rows = []
    for r in np.flatnonzero(cnt != k):
        if cnt[r] > k:
            tied = np.flatnonzero(op[r] == 0)
            excess = int(cnt[r]) - k
            if excess <= tied.size:
                mask[r, tied[tied.size - excess:]] = False
            else:
                mask[r] = False
                full_rows.append(r)
        else:
            mask[r] = False
            full_rows.append(r)

    rows, cols = np.nonzero(mask)
    vals = np.tanh(ALPHA * np.einsum("ij,ij->i", X[rows], W[cols])
                   ).astype(np.float32)
    out = np.zeros((N, N), np.float32)
    out[rows, cols] = np.maximum(vals, np.float32(0.0))
    for r in full_rows:
        out[r] = _row_reference(X, W, noise[r], r, k)

    out[np.arange(N), np.arange(N)] += np.float32(1.0)
    if _trace:
        return out, res
    return out


# revision 22
# speedup vs baseline: 5.4626x; 1.0409x over previous
"""Trainium2 Bass kernel for nn_graph_constructor (topk_masking).

Computes: adj = relu(tanh(3*(nv1@nv2.T - nv2@nv1.T))); per-row top-k of
(adj + 0.01*noise) masks adj; plus identity. Full [8192,8192] in/out.

Strategy (8 NeuronCores, row-sharded):
  - host: nv1/nv2 projections (tiny), pack X=[nv1|-nv2], W=[nv2|nv1] so the
    antisymmetric score block is ONE K=128 fp32 matmul per output tile.
  - device (per core, 1024 rows = 8 tiles of 128 partitions):
      PE:   a = X_blk @ W.T              (psum chunks)
      ACT:  tv = tanh(3*a); ns = 0.01*noise; final out' = relu(s - t_{k+1})
      DVE:  s = tv + ns; per-256-chunk top-8 candidates (InstMax);
            5 rounds max+match_replace on candidates -> (k+1)-th largest
      DMA:  noise in, out' rows out (memory-bound: ~64MiB/core)
    out'[i,j] = relu(s[i,j] - t_{k+1}[i]) is > 0 exactly on the top-k set
    (when t_k > t_{k+1}; boundary ties give < k positives -> host fallback).
  - host: mask = out' > 0; selected values recomputed exactly as
    tanh(3 * <X[r], W[c]>) (saturated tanh makes rounding immaterial);
    rare tie rows recomputed fully; add identity.

GpSimd is deliberately unused for elementwise work: measured ~123us per
[128,8192] tensor_scalar AND its SBUF traffic starves concurrent DVE ~10x.
"""

import numpy as np
from contextlib import ExitStack

import concourse.bass as bass
import concourse.bacc as bacc
import concourse.mybir as mybir
from concourse.tile import TileContext
from concourse.bass_utils import run_bass_kernel_spmd

ALPHA = 3.0
N = 8192
DIM = 64
CORES = 8
RPC = N // CORES          # rows per core
P = 128                   # partitions / tile rows
TILES = RPC // P          # row tiles per core
NBLK = 512                # matmul free-dim chunk (one PSUM bank)
PSB = 2048                # psum tile width (4 banks, 4 matmuls, 1 ACT pass)
CHUNK = 256               # stage-1 candidate chunk
NCH = N // CHUNK          # 32 chunks -> 256 candidates/row
F32 = mybir.dt.float32
NEG = -1.0e30

_prog_cache: dict = {}


def _build_program(k: int) -> bass.Bass:
    rounds = (k + 7) // 8              # extract the k-th largest
    last_col = (k - 1) % 8
    assert rounds * 8 <= NCH * 8

    nc = bacc.Bacc("TRN2", target_bir_lowering=False, debug=False,
                   num_devices=CORES)
    # lhsT block (xt, K=128 x RPC) + rhs (wt, K=128 x N) packed per tensor:
    # each matmul reads ONE tensor -> ONE dma semaphore (PE Matmult allows a
    # single sync wait). Split into wxa (xt + first wt chunk, small: first
    # matmuls start early) and wxb (xt again + remaining wt chunks).
    wxa_d = nc.dram_tensor("wxa", [P, RPC + PSB], F32, kind="ExternalInput").ap()
    wxb_d = nc.dram_tensor("wxb", [P, RPC + (N - PSB)], F32,
                           kind="ExternalInput").ap()
    nz_d = nc.dram_tensor("noise", [RPC, N], F32, kind="ExternalInput").ap()
    out_d = nc.dram_tensor("out", [RPC, N], F32, kind="ExternalOutput").ap()

    with TileContext(nc) as tc, ExitStack() as ctx:
        const_pool = ctx.enter_context(tc.tile_pool(name="const", bufs=1))
        a_pool = ctx.enter_context(tc.tile_pool(name="apool", bufs=4))
        b_pool = ctx.enter_context(tc.tile_pool(name="bpool", bufs=3))
        c_pool = ctx.enter_context(tc.tile_pool(name="cpool", bufs=2))
        m_pool = ctx.enter_context(tc.tile_pool(name="mpool", bufs=2))
        ps_pool = ctx.enter_context(
            tc.tile_pool(name="psum", bufs=2, space="PSUM"))

        wxa_sb = const_pool.tile([P, RPC + PSB], F32)
        nc.sync.dma_start(wxa_sb[:], wxa_d[:])
        wxb_sb = const_pool.tile([P, RPC + (N - PSB)], F32)
        nc.sync.dma_start(wxb_sb[:], wxb_d[:])

        for m in range(TILES):
            # pre-scaled noise (ns = 0.01*noise, scaled on host) for this
            # tile; buffer A is reused in place: ns -> s -> out'
            A = a_pool.tile([P, N], F32, tag="A")
            nc.sync.dma_start(A[:], nz_d[m * P:(m + 1) * P, :])

            # a -> tanh (psum -> sbuf bounce) -> add into A chunkwise
            for nb in range(N // PSB):
                src = wxa_sb if nb == 0 else wxb_sb
                base = RPC if nb == 0 else RPC + (nb - 1) * PSB
                ps = ps_pool.tile([P, PSB], F32, tag="ps")
                for h in range(PSB // NBLK):
                    off = base + h * NBLK
                    nc.tensor.matmul(ps[:, h * NBLK:(h + 1) * NBLK],
                                     src[:, m * P:(m + 1) * P],
                                     src[:, off:off + NBLK],
                                     start=True, stop=True)
                bc = b_pool.tile([P, PSB], F32, tag="bc")
                nc.scalar.activation(bc[:], ps[:],
                                     mybir.ActivationFunctionType.Tanh,
                                     bias=0.0, scale=ALPHA)
                # s chunk = ns chunk + tv chunk  (DVE, in place into A)
                nc.vector.tensor_add(A[:, nb * PSB:(nb + 1) * PSB],
                                     A[:, nb * PSB:(nb + 1) * PSB], bc[:])

            # stage 1: top-8 per 256-chunk -> 256 candidates
            cand = c_pool.tile([P, NCH * 8], F32, tag="cand")
            for c in range(NCH):
                nc.vector.max(cand[:, c * 8:(c + 1) * 8],
                              A[:, c * CHUNK:(c + 1) * CHUNK])

            # stage 2: iterative top-8 of candidates -> k-th largest
            maxs = m_pool.tile([P, rounds * 8], F32, tag="maxs")
            for r in range(rounds):
                ms = maxs[:, r * 8:(r + 1) * 8]
                nc.vector.max(ms, cand[:])
                if r < rounds - 1:
                    nc.vector.match_replace(cand[:], ms, cand[:], NEG)
            t_ap = maxs[:, rounds * 8 - 8 + last_col:rounds * 8 - 8 + last_col + 1]
            neg_t = m_pool.tile([P, 1], F32, tag="negt")
            nc.vector.tensor_scalar_mul(neg_t[:], t_ap, -1.0)

            # out' = s - t_k  (ACT Identity with per-partition bias; signed.
            # >0 above threshold, ==0 exactly on tied boundary, <0 below)
            # Split in halves so out-DMA starts before the whole tile is done.
            H = N // 2
            for h in range(2):
                nc.scalar.activation(A[:, h * H:(h + 1) * H],
                                     A[:, h * H:(h + 1) * H],
                                     mybir.ActivationFunctionType.Identity,
                                     bias=neg_t[:, 0:1], scale=1.0)
                nc.sync.dma_start(out_d[m * P:(m + 1) * P, h * H:(h + 1) * H],
                                  A[:, h * H:(h + 1) * H])
    nc.finalize()
    return nc


def get_program(k: int) -> bass.Bass:
    if k not in _prog_cache:
        _prog_cache[k] = _build_program(k)
    return _prog_cache[k]


def _host_nv(idx, emb1, emb2, lin1_w, lin1_b, lin2_w, lin2_b):
    idx = np.asarray(idx)
    e1 = np.asarray(emb1, dtype=np.float32)[idx]
    e2 = np.asarray(emb2, dtype=np.float32)[idx]
    nv1 = np.tanh(ALPHA * (e1 @ np.asarray(lin1_w, np.float32).T
                           + np.asarray(lin1_b, np.float32))).astype(np.float32)
    nv2 = np.tanh(ALPHA * (e2 @ np.asarray(lin2_w, np.float32).T
                           + np.asarray(lin2_b, np.float32))).astype(np.float32)
    return nv1, nv2


def _row_reference(X, W, noise_row, r, k):
    """Exact host recompute of one output row (pre-identity)."""
    a = (W @ X[r]).astype(np.float32)
    tv = np.tanh(ALPHA * a).astype(np.float32)
    adj = np.maximum(tv, np.float32(0.0))
    s = (adj + noise_row * np.float32(0.01)).astype(np.float32)
    order = np.argsort(-s, kind="stable")[:k]
    row = np.zeros(N, np.float32)
    row[order] = adj[order]
    return row


def kernel(idx, emb1, emb2, lin1_w, lin1_b, lin2_w, lin2_b, noise, k,
           _trace=False):
    k = int(k)
    noise = np.ascontiguousarray(np.asarray(noise, dtype=np.float32))
    # ns = 0.01 * noise, f32 RNE — bit-identical to the reference's scaling.
    # Done while sharding; device memory traffic is unchanged (it still
    # streams the full block), this just drops one on-chip elementwise pass.
    ns = noise * np.float32(0.01)
    nv1, nv2 = _host_nv(idx, emb1, emb2, lin1_w, lin1_b, lin2_w, lin2_b)

    X = np.concatenate([nv1, -nv2], axis=1).astype(np.float32)   # [N, 128]
    W = np.concatenate([nv2, nv1], axis=1).astype(np.float32)    # [N, 128]
    XT = np.ascontiguousarray(X.T)                               # [128, N]
    WT = np.ascontiguousarray(W.T)                               # [128, N]

    nc = get_program(k)
    in_maps = [{
        "wxa": np.ascontiguousarray(
            np.concatenate([XT[:, c * RPC:(c + 1) * RPC], WT[:, :PSB]], axis=1)),
        "wxb": np.ascontiguousarray(
            np.concatenate([XT[:, c * RPC:(c + 1) * RPC], WT[:, PSB:]], axis=1)),
        "noise": np.ascontiguousarray(ns[c * RPC:(c + 1) * RPC]),
    } for c in range(CORES)]

    res = run_bass_kernel_spmd(nc, in_maps, core_ids=list(range(CORES)),
                               trace=_trace)
    op = np.concatenate([res.results[c]["out"] for c in range(CORES)], axis=0)

    # --- host: mask = (s - t_k >= 0); ties sit exactly at 0 -> trim by
    # index (jax top_k keeps lowest indices); exact value reconstruction ---
    mask = op >= 0
    cnt = mask.sum(axis=1)
    full_rows = []
    for r in np.flatnonzero(cnt != k):
        if cnt[r] > k:
            tied = np.flatnonzero(op[r] == 0)
            excess = int(cnt[r]) - k
            if excess <= tied.size:
                mask[r, tied[tied.size - excess:]] = False
            else:
                mask[r] = False
                full_rows.append(r)
        else:
            mask[r] = False
            full_rows.append(r)

    rows, cols = np.nonzero(mask)
    vals = np.tanh(ALPHA * np.einsum("ij,ij->i", X[rows], W[cols])
                   ).astype(np.float32)
    out = np.zeros((N, N), np.float32)
    out[rows, cols] = np.maximum(vals, np.float32(0.0))
    for r in full_rows:
        out[r] = _row_reference(X, W, noise[r], r, k)

    out[np.arange(N), np.arange(N)] += np.float32(1.0)
    if _trace:
        return out, res
    return out


# revision 28
# speedup vs baseline: 5.8998x; 1.0800x over previous
"""Trainium2 Bass kernel for nn_graph_constructor (topk_masking).

Computes: adj = relu(tanh(3*(nv1@nv2.T - nv2@nv1.T))); per-row top-k of
(adj + 0.01*noise) masks adj; plus identity. Full [8192,8192] in/out.

Strategy (8 NeuronCores, row-sharded):
  - host: nv1/nv2 projections (tiny), pack X=[nv1|-nv2], W=[nv2|nv1] so the
    antisymmetric score block is ONE K=128 fp32 matmul per output tile.
  - device (per core, 1024 rows = 8 tiles of 128 partitions):
      PE:   a = X_blk @ W.T              (psum chunks)
      ACT:  tv = tanh(3*a); ns = 0.01*noise; final out' = relu(s - t_{k+1})
      DVE:  s = tv + ns; per-256-chunk top-8 candidates (InstMax);
            5 rounds max+match_replace on candidates -> (k+1)-th largest
      DMA:  noise in, out' rows out (memory-bound: ~64MiB/core)
    out'[i,j] = relu(s[i,j] - t_{k+1}[i]) is > 0 exactly on the top-k set
    (when t_k > t_{k+1}; boundary ties give < k positives -> host fallback).
  - host: mask = out' > 0; selected values recomputed exactly as
    tanh(3 * <X[r], W[c]>) (saturated tanh makes rounding immaterial);
    rare tie rows recomputed fully; add identity.

GpSimd is deliberately unused for elementwise work: measured ~123us per
[128,8192] tensor_scalar AND its SBUF traffic starves concurrent DVE ~10x.
"""

import numpy as np
from contextlib import ExitStack

import concourse.bass as bass
import concourse.bacc as bacc
import concourse.mybir as mybir
from concourse.tile import TileContext
from concourse.bass_utils import run_bass_kernel_spmd

ALPHA = 3.0
N = 8192
DIM = 64
CORES = 8
RPC = N // CORES          # rows per core
P = 128                   # partitions / tile rows
TILES = RPC // P          # row tiles per core
NBLK = 512                # matmul free-dim chunk (one PSUM bank)
PSB = 2048                # psum tile width (4 banks, 4 matmuls, 1 ACT pass)
CHUNK = 256               # stage-1 candidate chunk
NCH = N // CHUNK          # 32 chunks -> 256 candidates/row
F32 = mybir.dt.float32
BF16 = mybir.dt.bfloat16
NEG = -1.0e30

_prog_cache: dict = {}


def _build_program(k: int) -> bass.Bass:
    rounds = (k + 7) // 8              # extract the k-th largest
    last_col = (k - 1) % 8
    assert rounds * 8 <= NCH * 8

    nc = bacc.Bacc("TRN2", target_bir_lowering=False, debug=False,
                   num_devices=CORES)
    # lhsT block (xt, K=128 x RPC) + rhs (wt, K=128 x N) packed per tensor:
    # each matmul reads ONE tensor -> ONE dma semaphore (PE Matmult allows a
    # single sync wait). Split into wxa (xt + first wt chunk, small: first
    # matmuls start early) and wxb (xt again + remaining wt chunks).
    wxa_d = nc.dram_tensor("wxa", [P, RPC + PSB], F32, kind="ExternalInput").ap()
    wxb_d = nc.dram_tensor("wxb", [P, RPC + (N - PSB)], F32,
                           kind="ExternalInput").ap()
    nz_d = nc.dram_tensor("noise", [RPC, N], F32, kind="ExternalInput").ap()
    # out carries only sign/zero info (host reconstructs values): bf16
    # halves the write traffic; sign and exact-zero survive the rounding.
    out_d = nc.dram_tensor("out", [RPC, N], BF16, kind="ExternalOutput").ap()

    with TileContext(nc) as tc, ExitStack() as ctx:
        const_pool = ctx.enter_context(tc.tile_pool(name="const", bufs=1))
        a_pool = ctx.enter_context(tc.tile_pool(name="apool", bufs=3))
        b_pool = ctx.enter_context(tc.tile_pool(name="bpool", bufs=3))
        o_pool = ctx.enter_context(tc.tile_pool(name="opool", bufs=2))
        c_pool = ctx.enter_context(tc.tile_pool(name="cpool", bufs=2))
        m_pool = ctx.enter_context(tc.tile_pool(name="mpool", bufs=2))
        ps_pool = ctx.enter_context(
            tc.tile_pool(name="psum", bufs=2, space="PSUM"))

        wxa_sb = const_pool.tile([P, RPC + PSB], F32)
        nc.sync.dma_start(wxa_sb[:], wxa_d[:])
        wxb_sb = const_pool.tile([P, RPC + (N - PSB)], F32)
        nc.sync.dma_start(wxb_sb[:], wxb_d[:])

        for m in range(TILES):
            # pre-scaled noise (ns = 0.01*noise, scaled on host) for this
            # tile; buffer A is reused in place: ns -> s. Halved DMA so the
            # first add chunks can start sooner.
            A = a_pool.tile([P, N], F32, tag="A")
            nc.sync.dma_start(A[:, :N // 2], nz_d[m * P:(m + 1) * P, :N // 2])
            nc.sync.dma_start(A[:, N // 2:], nz_d[m * P:(m + 1) * P, N // 2:])

            # a -> tanh (psum -> sbuf bounce) -> add into A chunkwise
            for nb in range(N // PSB):
                src = wxa_sb if nb == 0 else wxb_sb
                base = RPC if nb == 0 else RPC + (nb - 1) * PSB
                ps = ps_pool.tile([P, PSB], F32, tag="ps")
                for h in range(PSB // NBLK):
                    off = base + h * NBLK
                    nc.tensor.matmul(ps[:, h * NBLK:(h + 1) * NBLK],
                                     src[:, m * P:(m + 1) * P],
                                     src[:, off:off + NBLK],
                                     start=True, stop=True)
                bc = b_pool.tile([P, PSB], F32, tag="bc")
                nc.scalar.activation(bc[:], ps[:],
                                     mybir.ActivationFunctionType.Tanh,
                                     bias=0.0, scale=ALPHA)
                # s chunk = ns chunk + tv chunk  (DVE, in place into A)
                nc.vector.tensor_add(A[:, nb * PSB:(nb + 1) * PSB],
                                     A[:, nb * PSB:(nb + 1) * PSB], bc[:])

            # stage 1: top-8 per 256-chunk -> 256 candidates
            cand = c_pool.tile([P, NCH * 8], F32, tag="cand")
            for c in range(NCH):
                nc.vector.max(cand[:, c * 8:(c + 1) * 8],
                              A[:, c * CHUNK:(c + 1) * CHUNK])

            # stage 2: iterative top-8 of candidates -> k-th largest
            maxs = m_pool.tile([P, rounds * 8], F32, tag="maxs")
            for r in range(rounds):
                ms = maxs[:, r * 8:(r + 1) * 8]
                nc.vector.max(ms, cand[:])
                if r < rounds - 1:
                    nc.vector.match_replace(cand[:], ms, cand[:], NEG)
            t_ap = maxs[:, rounds * 8 - 8 + last_col:rounds * 8 - 8 + last_col + 1]
            neg_t = m_pool.tile([P, 1], F32, tag="negt")
            nc.vector.tensor_scalar_mul(neg_t[:], t_ap, -1.0)

            # out' = s - t_k  (ACT Identity with per-partition bias; signed.
            # >0 above threshold, ==0 exactly on tied boundary, <0 below)
            # Split in halves so out-DMA starts before the whole tile is done.
            H = N // 2
            O = o_pool.tile([P, N], BF16, tag="O")
            for h in range(2):
                nc.scalar.activation(O[:, h * H:(h + 1) * H],
                                     A[:, h * H:(h + 1) * H],
                                     mybir.ActivationFunctionType.Identity,
                                     bias=neg_t[:, 0:1], scale=1.0)
                nc.sync.dma_start(out_d[m * P:(m + 1) * P, h * H:(h + 1) * H],
                                  O[:, h * H:(h + 1) * H])
    nc.finalize()
    return nc


def get_program(k: int) -> bass.Bass:
    if k not in _prog_cache:
        _prog_cache[k] = _build_program(k)
    return _prog_cache[k]


def _host_nv(idx, emb1, emb2, lin1_w, lin1_b, lin2_w, lin2_b):
    idx = np.asarray(idx)
    e1 = np.asarray(emb1, dtype=np.float32)[idx]
    e2 = np.asarray(emb2, dtype=np.float32)[idx]
    nv1 = np.tanh(ALPHA * (e1 @ np.asarray(lin1_w, np.float32).T
                           + np.asarray(lin1_b, np.float32))).astype(np.float32)
    nv2 = np.tanh(ALPHA * (e2 @ np.asarray(lin2_w, np.float32).T
                           + np.asarray(lin2_b, np.float32))).astype(np.float32)
    return nv1, nv2


def _row_reference(X, W, noise_row, r, k):
    """Exact host recompute of one output row (pre-identity)."""
    a = (W @ X[r]).astype(np.float32)
    tv = np.tanh(ALPHA * a).astype(np.float32)
    adj = np.maximum(tv, np.float32(0.0))
    s = (adj + noise_row * np.float32(0.01)).astype(np.float32)
    order = np.argsort(-s, kind="stable")[:k]
    row = np.zeros(N, np.float32)
    row[order] = adj[order]
    return row


def kernel(idx, emb1, emb2, lin1_w, lin1_b, lin2_w, lin2_b, noise, k,
           _trace=False):
    k = int(k)
    noise = np.ascontiguousarray(np.asarray(noise, dtype=np.float32))
    # ns = 0.01 * noise, f32 RNE — bit-identical to the reference's scaling.
    # Done while sharding; device memory traffic is unchanged (it still
    # streams the full block), this just drops one on-chip elementwise pass.
    ns = noise * np.float32(0.01)
    nv1, nv2 = _host_nv(idx, emb1, emb2, lin1_w, lin1_b, lin2_w, lin2_b)

    X = np.concatenate([nv1, -nv2], axis=1).astype(np.float32)   # [N, 128]
    W = np.concatenate([nv2, nv1], axis=1).astype(np.float32)    # [N, 128]
    XT = np.ascontiguousarray(X.T)                               # [128, N]
    WT = np.ascontiguousarray(W.T)                               # [128, N]

    nc = get_program(k)
    in_maps = [{
        "wxa": np.ascontiguousarray(
            np.concatenate([XT[:, c * RPC:(c + 1) * RPC], WT[:, :PSB]], axis=1)),
        "wxb": np.ascontiguousarray(
            np.concatenate([XT[:, c * RPC:(c + 1) * RPC], WT[:, PSB:]], axis=1)),
        "noise": np.ascontiguousarray(ns[c * RPC:(c + 1) * RPC]),
    } for c in range(CORES)]

    res = run_bass_kernel_spmd(nc, in_maps, core_ids=list(range(CORES)),
                               trace=_trace)
    op = np.concatenate([res.results[c]["out"] for c in range(CORES)],
                        axis=0)  # bf16, sign/zero of s - t_k

    # --- host: mask = (s - t_k >= 0); ties sit exactly at 0 -> trim by
    # index (jax top_k keeps lowest indices); exact value reconstruction ---
    mask = op >= 0
    cnt = mask.sum(axis=1)
    full_rows = []
    for r in np.flatnonzero(cnt != k):
        if cnt[r] > k:
            tied = np.flatnonzero(op[r] == 0)
            excess = int(cnt[r]) - k
            if excess <= tied.size:
                mask[r, tied[tied.size - excess:]] = False
            else:
                mask[r] = False
                full_rows.append(r)
        else:
            mask[r] = False
            full_rows.append(r)

    rows, cols = np.nonzero(mask)
    vals = np.tanh(ALPHA * np.einsum("ij,ij->i", X[rows], W[cols])
                   ).astype(np.float32)
    out = np.zeros((N, N), np.float32)
    out[rows, cols] = np.maximum(vals, np.float32(0.0))
    for r in full_rows:
        out[r] = _row_reference(X, W, noise[r], r, k)

    out[np.arange(N), np.arange(N)] += np.float32(1.0)
    if _trace:
        return out, res
    return out
